# revision 9
# baseline (speedup 1.0000x reference)
"""Trainium2 Bass kernel for nn_FEELModel (TreeLSTM + triplet embedding model).

Strategy:
- Data-parallel over batch B=512 across 8 NeuronCores (64 rows/core).
- fp8(e4m3) embedding gather (table pre-scaled by S=512 on host; the scale is
  folded into Wioux/Wfx and the triplet-dot epilogue) -> halves gather bytes.
- The gather (55.8k 512B rows/core) is the entire critical path; it is
  descriptor-path-bound (~8.5 ns/desc per SWDGE queue), so gathers round-robin
  all 4 SWDGE queues with ~5 calls in flight each (640 descs/call keeps the
  1024-desc ring from bubbling). Everything else overlaps under it.
- fp8 membership matrices (preloaded whole), bf16 weights, bf16 tail GEMMs
  (f32 matmul is 4 cycles/row on PE; bf16 is 1).
- TreeLSTM + similarity tail is emitted between seq and attr pooling so its
  PE work overlaps the attr gather DMA.
- Adaptive parity capacities: computed from the actual token data at program
  build time (program cached per capacity), eliminating the 8-sigma padding.
- wsum (colsum of Wwp) computed on host.
- REPS repeats the whole compute body for amplified wall-clock timing.
"""
import sys

if "/opt/trn_rl_repo" not in sys.path:
    sys.path.insert(0, "/opt/trn_rl_repo")

from contextlib import ExitStack

import numpy as np

import concourse.bass as bass
import concourse.bacc as bacc
import concourse.mybir as mybir
import concourse.tile as tile
from concourse.bass_utils import run_bass_kernel_spmd

F32 = mybir.dt.float32
BF16 = mybir.dt.bfloat16
FP8 = mybir.dt.float8e4
I16 = mybir.dt.int16
AF = mybir.ActivationFunctionType
ALU = mybir.AluOpType

# Full-size problem config (hardcoded; harness contract).
B, NC_CORES, L, LQ, V, D, M, H, O = 512, 8, 64, 128, 50000, 512, 512, 256, 30
S_EMB = 512.0  # fp8 storage scale for emb (power of two)
SPC = 5   # gather slabs (128 rows each) per dma_gather call; 640 descs/call
          # leaves headroom in the 1024-desc SWDGE ring so back-to-back calls
          # on one queue overlap (SPC=8 fills the ring exactly and bubbles;
          # SPC>8 wedges the device).
GBUFS = 28  # gather buffers in flight (~7 per queue)
REPS = 1   # repeat whole compute body (amplified timing)
MODE = "full"  # full | gp (no tail) | g (gathers only)
QUEUES = 4  # SWDGE queues for gathers, round-robin (4 = ucode max; the
            # descriptor path is per-queue serial at ~8.5ns/desc, so 4 queues
            # quadruple gather throughput)
DR = False  # DoubleRow fp8 perf mode for pooling matmuls (slab pairs)
SCRATCH = 16384  # SWDGE descriptor carveout bytes
SINGLE_PACKET = True  # dma_gather single_packet flag

ATTR_KEYS = ["q_v", "q_a0", "n_a0", "q_a1", "n_a1", "q_a2", "n_a2"]
SEQ_KEYS = ["query", "pos", "neg"]


def _seq_window(Bc, LQ):
    span = max(1, 256 // LQ)  # batch rows spanned by one 128-position slab
    return span, min(Bc, 2 * span + 12)


def _seq_base(s, span, W, Bc, pair=False):
    if pair:  # DoubleRow: both slabs of an even-aligned pair share a window
        s = (s // 2) * 2
    return int(np.clip(span * s - (W - span) // 2, 0, Bc - W))


def _parity_caps(inputs):
    """Max parity count over (stream, core, parity), rounded to 128, for the
    attr and seq stream families."""
    caps = {}
    for fam, keys, length in (("a", ATTR_KEYS, L), ("s", SEQ_KEYS, LQ)):
        mx = 0
        for key in keys:
            toks = np.asarray(inputs[key]).reshape(NC_CORES, -1)
            ev = (toks % 2 == 0).sum(axis=1)
            mx = max(mx, int(ev.max()), int((toks.shape[1] - ev).max()))
        caps[fam] = ((mx + 127) // 128) * 128
    return caps["a"], caps["s"]


def build_program(Bc, L, LQ, V, D, M, H, O, CAP_A, CAP_S, reps, mode="full"):
    DC = D // 128
    MC = M // 128
    HC = H // 128
    NPT = 4 * Bc          # pooled cols per tree (4b+node layout)
    LB = 3 * Bc
    PS_T = 256            # per-tree column stride in f psum
    SL_A = CAP_A // 128
    SL_S = CAP_S // 128
    span, WB = _seq_window(Bc, LQ)
    assert NPT <= 256 and 4 * WB <= NPT

    nc = bacc.Bacc("TRN2", target_bir_lowering=False, debug=False,
                   num_swdge_queues=QUEUES, dynamic_dma_scratch_size=SCRATCH)

    emb_d = nc.dram_tensor("emb", (V, D), FP8, kind="ExternalInput")
    idx_d = nc.dram_tensor("idx", (128, (3 * SL_S + 7 * SL_A) * 2 * 8), I16, kind="ExternalInput")
    memb_s_d = nc.dram_tensor("memb_s", (128, 3 * 2 * SL_S, 4 * WB), FP8, kind="ExternalInput")
    memb_a_d = nc.dram_tensor("memb_a", (128, 7 * 2 * SL_A, Bc), FP8, kind="ExternalInput")
    Wioux_d = nc.dram_tensor("Wioux", (D, 3 * M), BF16, kind="ExternalInput")
    Wiouh_d = nc.dram_tensor("Wiouh", (M, 3 * M), BF16, kind="ExternalInput")
    Wfx_d = nc.dram_tensor("Wfx", (D, M), BF16, kind="ExternalInput")
    Wfh_d = nc.dram_tensor("Wfh", (M, M), BF16, kind="ExternalInput")
    Wwh_d = nc.dram_tensor("Wwh", (M, H), BF16, kind="ExternalInput")
    wsum_d = nc.dram_tensor("wsum", (H,), F32, kind="ExternalInput")
    biou_d = nc.dram_tensor("biou", (3 * M,), F32, kind="ExternalInput")
    bf_d = nc.dram_tensor("bf", (M,), F32, kind="ExternalInput")
    bwh_d = nc.dram_tensor("bwh", (H,), F32, kind="ExternalInput")
    out_d = nc.dram_tensor("out", (Bc,), F32, kind="ExternalOutput")

    emb_pairs = emb_d[:].rearrange("(v two) d -> v two d", two=2)

    with tile.TileContext(nc) as tc, ExitStack() as ctx:
        sb = ctx.enter_context(tc.tile_pool(name="sb", bufs=1))
        ps = ctx.enter_context(tc.tile_pool(name="ps", bufs=1, space="PSUM"))

        # ---- loads ----
        idx_t = sb.tile([128, idx_d.shape[1]], I16)
        nc.sync.dma_start(idx_t[:], idx_d[:])
        wioux_t = sb.tile([128, DC, 3 * M], BF16)
        nc.sync.dma_start(wioux_t[:], Wioux_d[:].rearrange("(c p) m -> p c m", p=128))
        wiouh_t = sb.tile([128, MC, 2 * M], BF16)
        nc.sync.dma_start(wiouh_t[:, :, :M], Wiouh_d[:, 0:M].rearrange("(c p) m -> p c m", p=128))
        nc.sync.dma_start(wiouh_t[:, :, M:], Wiouh_d[:, 2 * M:3 * M].rearrange("(c p) m -> p c m", p=128))
        wfx_t = sb.tile([128, DC, M], BF16)
        nc.sync.dma_start(wfx_t[:], Wfx_d[:].rearrange("(c p) m -> p c m", p=128))
        wfh_t = sb.tile([128, MC, M], BF16)
        nc.sync.dma_start(wfh_t[:], Wfh_d[:].rearrange("(c p) m -> p c m", p=128))
        wwh_t = sb.tile([128, MC, H], BF16)
        nc.sync.dma_start(wwh_t[:], Wwh_d[:].rearrange("(c p) m -> p c m", p=128))
        wsum_t = sb.tile([128, HC], F32)
        nc.sync.dma_start(wsum_t[:], wsum_d[:].rearrange("(c p) -> p c", p=128))
        biou_t = sb.tile([128, 3 * MC], F32)
        nc.sync.dma_start(biou_t[:], biou_d[:].rearrange("(c p) -> p c", p=128))
        bf_t = sb.tile([128, MC], F32)
        nc.sync.dma_start(bf_t[:], bf_d[:].rearrange("(c p) -> p c", p=128))
        bwh_t = sb.tile([128, HC], F32)
        nc.sync.dma_start(bwh_t[:], bwh_d[:].rearrange("(c p) -> p c", p=128))

        ones_t = sb.tile([128, 1], F32)
        nc.vector.memset(ones_t[:], 1.0)
        zeros8_t = sb.tile([128, 256], FP8)
        nc.vector.memset(zeros8_t[:], 0.0)

        # Preload ALL membership matrices (one DMA each; ~28KB/partition).
        membs_t = sb.tile([128, 3 * 2 * SL_S, 4 * WB], FP8)
        nc.sync.dma_start(membs_t[:], memb_s_d[:])
        memba_t = sb.tile([128, 7 * 2 * SL_A, Bc], FP8)
        nc.sync.dma_start(memba_t[:], memb_a_d[:])

        # ---- gather + pooling ----
        # idx column layout: streams [seq0,seq1,seq2,attr0..6], within a stream
        # parity 0 then parity 1; cols per (stream, parity) = CAP/16.
        state = {"col": 0, "q": 0}

        def pool_stream(is_seq, pool_ps, memb_t, slab_base, nsl, out_cols_fn):
            for e in range(2):
                s0 = 0
                while s0 < nsl:
                    ns = min(SPC, nsl - s0)
                    c0 = state["col"]
                    state["col"] += ns * 8
                    g = sb.tile([128, SPC, D], FP8, name="g", tag="g", bufs=GBUFS)
                    so = slab_base + e * nsl + s0
                    nc.gpsimd.dma_gather(
                        out_ap=g[:, :ns, :],
                        in_ap=emb_pairs[:, e, :],
                        idxs_ap=idx_t[:, c0:c0 + ns * 8],
                        num_idxs=ns * 128,
                        num_idxs_reg=ns * 128,
                        elem_size=D,
                        elem_step=2 * D,
                        queue_num=state["q"] % QUEUES,
                        single_packet=SINGLE_PACKET,
                    )
                    state["q"] += 1
                    if mode != "g":
                        j = 0
                        while j < ns:
                            s = s0 + j
                            first = (e == 0 and s == 0)
                            if DR and j + 1 < ns:
                                last = (e == 1 and s + 1 == nsl - 1)
                                for c in range(DC):
                                    nc.tensor.matmul(
                                        out=out_cols_fn(pool_ps, c, s),
                                        lhsT=g[:, j:j + 2, c * 128:(c + 1) * 128],
                                        rhs=memb_t[:, so + j:so + j + 2, :],
                                        start=(False if is_seq else first),
                                        stop=last,
                                        perf_mode=mybir.MatmulPerfMode.DoubleRow,
                                        skip_group_check=True,
                                    )
                                j += 2
                            else:
                                last = (e == 1 and s == nsl - 1)
                                for c in range(DC):
                                    nc.tensor.matmul(
                                        out=out_cols_fn(pool_ps, c, s),
                                        lhsT=g[:, j, c * 128:(c + 1) * 128],
                                        rhs=memb_t[:, so + j, :],
                                        start=(False if is_seq else first),
                                        stop=last,
                                        skip_group_check=True,
                                    )
                                j += 1
                    s0 += ns

        for _rep in range(reps):
            state["col"] = 0
            # seq streams first (their results gate the TreeLSTM GEMMs)
            xT3 = sb.tile([128, DC, 3 * NPT], BF16)
            for t in range(3):
                pool_ps = ps.tile([128, DC, NPT], F32, name="pool_ps", tag="pool")
                if mode != "g":
                    for c in range(DC):  # zero-prelude: clear has_written + zero cols
                        nc.tensor.matmul(out=pool_ps[:, c, :], lhsT=zeros8_t[:, :128],
                                         rhs=zeros8_t[:, :NPT], start=True, stop=False,
                                         skip_group_check=True)

                def seq_cols(pp, c, s):
                    base = _seq_base(s, span, WB, Bc, pair=DR)
                    return pp[:, c, :].rearrange("p (b n) -> p b n", n=4)[:, base:base + WB, :]

                pool_stream(True, pool_ps, membs_t, t * 2 * SL_S, SL_S, seq_cols)
                if mode != "g":
                    nc.vector.tensor_copy(xT3[:, :, t * NPT:(t + 1) * NPT], pool_ps[:])

            # TreeLSTM + similarity-hinge tail (only needs xT3) is emitted
            # BEFORE attr pooling so its PE work overlaps the attr gather DMA.
            if mode == "full":
                hinge = _run_tree_tail(**locals())

            attr_sb = sb.tile([128, 7, DC, Bc], F32)
            for k in range(7):
                pool_psa = ps.tile([128, DC, Bc], F32, name="pool_psa", tag="poolA")

                def attr_cols(pp, c, s):
                    return pp[:, c, :]

                pool_stream(False, pool_psa, memba_t, k * 2 * SL_A, SL_A, attr_cols)
                if mode != "g":
                    nc.vector.tensor_copy(attr_sb[:, k], pool_psa[:])

            if mode == "full":
                _run_dots_tail(**locals())
            else:
                fin = sb.tile([1, Bc], F32, name="fin", tag="fin")
                nc.vector.memset(fin[:], 0.0)
                nc.sync.dma_start(out_d[None, :], fin[:1, :])

    nc.compile()
    return nc


def _run_tree_tail(nc, tc, sb, ps, Bc, DC, MC, HC, NPT, LB, PS_T, xT3,
                   wioux_t, wiouh_t, wfx_t, wfh_t, wwh_t, biou_t, bf_t, bwh_t,
                   wsum_t, M, **_kw):
    # ---- TreeLSTM leaves ----
    # col layouts: xT3 per tree: 4b+node; leaves (cL/hL): 3b+j; root (cr): t*Bc+b.
    cL = sb.tile([128, MC, 3 * LB], BF16)
    hL = sb.tile([128, MC, 3 * LB], BF16)
    for t in range(3):
        xleaf = xT3[:, :, t * NPT:(t + 1) * NPT].rearrange("p c (b n) -> p c b n", n=4)[:, :, :, 0:3]
        for r in range(2):  # mc rounds {0,1},{2,3}
            iou_ps = ps.tile([128, 6, 256], F32, name="iou_ps", tag="psA")
            for i, mc in enumerate([2 * r, 2 * r + 1]):
                for part in range(3):  # i, o, u
                    for kc in range(DC):
                        nc.tensor.matmul(
                            out=iou_ps[:, part * 2 + i, :LB],
                            lhsT=wioux_t[:, kc, (part * MC + mc) * 128:(part * MC + mc + 1) * 128],
                            rhs=xleaf[:, kc],
                            start=(kc == 0), stop=(kc == DC - 1),
                        )
            ti = sb.tile([128, LB], BF16, name="ti", tag="ti")
            tu = sb.tile([128, LB], BF16, name="tu", tag="tu")
            to = sb.tile([128, LB], BF16, name="to", tag="to")
            for i, mc in enumerate([2 * r, 2 * r + 1]):
                nc.scalar.activation(ti[:], iou_ps[:, i, :LB], AF.Sigmoid, bias=biou_t[:, mc:mc + 1])
                nc.scalar.activation(to[:], iou_ps[:, 2 + i, :LB], AF.Sigmoid, bias=biou_t[:, MC + mc:MC + mc + 1])
                nc.scalar.activation(tu[:], iou_ps[:, 4 + i, :LB], AF.Tanh, bias=biou_t[:, 2 * MC + mc:2 * MC + mc + 1])
                nc.vector.tensor_mul(cL[:, mc, t * LB:(t + 1) * LB], ti[:], tu[:])
                nc.scalar.activation(ti[:], cL[:, mc, t * LB:(t + 1) * LB], AF.Tanh)
                nc.vector.tensor_mul(hL[:, mc, t * LB:(t + 1) * LB], to[:], ti[:])

    # ---- root ----
    hs = sb.tile([128, MC, 3 * Bc], BF16)  # cols t*Bc+b
    for t in range(3):
        for c in range(MC):
            hj = hL[:, c, t * LB:(t + 1) * LB].rearrange("p (b j) -> p b j", j=3)
            nc.vector.tensor_add(hs[:, c, t * Bc:(t + 1) * Bc], hj[:, :, 0], hj[:, :, 1])
            nc.vector.tensor_add(hs[:, c, t * Bc:(t + 1) * Bc],
                                 hs[:, c, t * Bc:(t + 1) * Bc], hj[:, :, 2])

    xroot = xT3[:, :, :].rearrange("p c (t b n) -> p c t b n", t=3, n=4)[:, :, :, :, 3]

    # f gates (mc rounds of 2), g = Wfx @ x_root
    f_sb = sb.tile([128, MC, 3 * LB], BF16)
    g_ps = ps.tile([128, MC, 256], F32, name="g_ps", tag="psB")
    for mc in range(MC):
        for kc in range(DC):
            nc.tensor.matmul(
                out=g_ps[:, mc, :3 * Bc],
                lhsT=wfx_t[:, kc, mc * 128:(mc + 1) * 128],
                rhs=xroot[:, kc],
                start=(kc == 0), stop=(kc == DC - 1),
            )
    g_sb = sb.tile([128, MC, 3 * Bc], BF16)
    nc.vector.tensor_copy(g_sb[:], g_ps[:, :, :3 * Bc])
    for r in range(2):
        f_ps = ps.tile([128, 2, 3 * PS_T], F32, name="f_ps", tag="psA")
        for i, mc in enumerate([2 * r, 2 * r + 1]):
            for t in range(3):
                for kc in range(MC):
                    nc.tensor.matmul(
                        out=f_ps[:, i, t * PS_T:t * PS_T + LB],
                        lhsT=wfh_t[:, kc, mc * 128:(mc + 1) * 128],
                        rhs=hL[:, kc, t * LB:(t + 1) * LB],
                        start=(kc == 0), stop=(kc == MC - 1),
                    )
        for i, mc in enumerate([2 * r, 2 * r + 1]):
            nc.vector.tensor_add(
                f_sb[:, mc, :].rearrange("p (t b j) -> p t b j", t=3, j=3),
                f_ps[:, i, :].rearrange("p (t x) -> p t x", t=3)[:, :, :LB].rearrange("p t (b j) -> p t b j", j=3),
                g_sb[:, mc, :].rearrange("p (t b) -> p t b", t=3)[:, :, :, None].to_broadcast([128, 3, Bc, 3]),
            )
            nc.scalar.activation(f_sb[:, mc, :], f_sb[:, mc, :], AF.Sigmoid, bias=bf_t[:, mc:mc + 1])

    # root i,u + c_root
    cr = sb.tile([128, MC, 3 * Bc], BF16)
    ri = sb.tile([128, 3 * Bc], BF16, name="ri", tag="ti")
    ru = sb.tile([128, 3 * Bc], BF16, name="ru", tag="tu")
    for r in range(2):
        riou_ps = ps.tile([128, 4, 256], F32, name="riou_ps", tag="psA")
        for i, mc in enumerate([2 * r, 2 * r + 1]):
            for half, wof in ((0, 0), (1, M)):
                for kc in range(DC):
                    nc.tensor.matmul(
                        out=riou_ps[:, half * 2 + i, :3 * Bc],
                        lhsT=(wioux_t[:, kc, mc * 128:(mc + 1) * 128] if half == 0
                              else wioux_t[:, kc, (2 * MC + mc) * 128:(2 * MC + mc + 1) * 128]),
                        rhs=xroot[:, kc],
                        start=(kc == 0), stop=False,
                    )
                for kc in range(MC):
                    nc.tensor.matmul(
                        out=riou_ps[:, half * 2 + i, :3 * Bc],
                        lhsT=wiouh_t[:, kc, wof + mc * 128:wof + (mc + 1) * 128],
                        rhs=hs[:, kc, :],
                        start=False, stop=(kc == MC - 1),
                    )
        for i, mc in enumerate([2 * r, 2 * r + 1]):
            nc.scalar.activation(ri[:], riou_ps[:, i, :3 * Bc], AF.Sigmoid, bias=biou_t[:, mc:mc + 1])
            nc.scalar.activation(ru[:], riou_ps[:, 2 + i, :3 * Bc], AF.Tanh, bias=biou_t[:, 2 * MC + mc:2 * MC + mc + 1])
            nc.vector.tensor_mul(cr[:, mc, :], ri[:], ru[:])
    for c in range(MC):
        fc_c = sb.tile([128, 3 * LB], BF16, name="fc_c", tag="to")
        nc.vector.tensor_mul(fc_c[:], f_sb[:, c, :], cL[:, c, :])
        for j in range(3):
            nc.vector.tensor_add(
                cr[:, c, :].rearrange("p (t b) -> p t b", t=3),
                cr[:, c, :].rearrange("p (t b) -> p t b", t=3),
                fc_c[:].rearrange("p (t b j) -> p t b j", t=3, j=3)[:, :, :, j],
            )

    # ---- similarity ----
    zq = sb.tile([128, DC, 2 * Bc], BF16)
    for c in range(MC):
        nc.vector.tensor_mul(
            zq[:, c, :].rearrange("p (r b) -> p r b", r=2),
            cr[:, c, 0:Bc][:, None, :].to_broadcast([128, 2, Bc]),
            cr[:, c, Bc:3 * Bc].rearrange("p (r b) -> p r b", r=2),
        )
    sh_ps = ps.tile([128, HC, 128], F32, name="sh_ps", tag="pool")
    for hc in range(HC):
        for kc in range(MC):
            nc.tensor.matmul(
                out=sh_ps[:, hc, :2 * Bc],
                lhsT=wwh_t[:, kc, hc * 128:(hc + 1) * 128],
                rhs=zq[:, kc, :],
                start=(kc == 0), stop=(kc == MC - 1),
            )
    sig_sb = sb.tile([128, HC, 2 * Bc], F32)
    for hc in range(HC):
        nc.scalar.activation(sig_sb[:, hc, :], sh_ps[:, hc, :2 * Bc], AF.Sigmoid, bias=bwh_t[:, hc:hc + 1])
    ab_ps = ps.tile([1, 2 * Bc], F32, name="ab_ps", tag="pool")
    for hc in range(HC):
        nc.tensor.matmul(
            out=ab_ps[:, :], lhsT=wsum_t[:, hc:hc + 1], rhs=sig_sb[:, hc, :],
            start=(hc == 0), stop=(hc == HC - 1),
        )
    ab_sb = sb.tile([1, 2 * Bc], F32)
    nc.vector.tensor_copy(ab_sb[:], ab_ps[:1, :])
    dab = sb.tile([1, Bc], F32)
    nc.vector.tensor_sub(dab[:], ab_sb[:1, Bc:2 * Bc], ab_sb[:1, 0:Bc])
    hinge = sb.tile([1, Bc], F32)
    nc.scalar.activation(hinge[:], dab[:], AF.Relu, bias=1.0)
    return hinge


def _run_dots_tail(nc, sb, ps, Bc, DC, attr_sb, ones_t, hinge, out_d, **_kw):
    # ---- triplet losses ----
    dt = sb.tile([128, DC, Bc], F32, name="dt", tag="dt")
    mt2 = sb.tile([128, DC, Bc], F32, name="mt2", tag="mt2")
    dots_ps = ps.tile([1, 3, Bc], F32, name="dots_ps", tag="pool")
    for k in range(3):
        nc.vector.tensor_sub(dt[:], attr_sb[:, 1 + 2 * k], attr_sb[:, 2 + 2 * k])
        nc.vector.tensor_mul(mt2[:], attr_sb[:, 0], dt[:])
        for c in range(DC):
            nc.tensor.matmul(
                out=dots_ps[:1, k, :], lhsT=ones_t[:], rhs=mt2[:, c, :],
                start=(c == 0), stop=(c == DC - 1),
            )
    loss3 = sb.tile([1, 3, Bc], F32)
    nc.scalar.activation(loss3[:1, :, :], dots_ps[:1, :, :], AF.Relu, bias=1.0,
                         scale=-1.0 / (S_EMB * S_EMB))
    loss = sb.tile([1, Bc], F32)
    nc.vector.tensor_add(loss[:], loss3[:1, 0, :], loss3[:1, 1, :])
    nc.vector.tensor_add(loss[:], loss[:], loss3[:1, 2, :])

    fin = sb.tile([1, Bc], F32)
    nc.vector.tensor_add(fin[:], loss[:], hinge[:])
    nc.sync.dma_start(out_d[None, :], fin[:1, :])


_PROG_CACHE = {}


def _get_program(*args):
    key = (*args, MODE, QUEUES, DR, SCRATCH, SPC, GBUFS, SINGLE_PACKET)
    if key not in _PROG_CACHE:
        _PROG_CACHE[key] = build_program(*args, mode=MODE)
    return _PROG_CACHE[key]


def _wrap_idx(flat):
    """[n] -> [128, n/16] int16 wrapped (flat i = s*16 + p), replicated x8."""
    w = flat.reshape(-1, 16).T
    return np.tile(w, (8, 1)).astype(np.int16)


def _prep_core_inputs(inputs, ci, Bc, L, LQ, CAP_A, CAP_S):
    sl = slice(ci * Bc, (ci + 1) * Bc)
    SL_A, SL_S = CAP_A // 128, CAP_S // 128
    span, WB = _seq_window(Bc, LQ)
    npn = LQ // 4

    import ml_dtypes
    FP8NP = ml_dtypes.float8_e4m3
    idx_cols = []
    memb_s = np.zeros((128, 3 * 2 * SL_S, 4 * WB), FP8NP)
    memb_a = np.zeros((128, 7 * 2 * SL_A, Bc), FP8NP)

    def add_stream(tokens, cap, memb, slab_base, col_fn, w):
        nsl = cap // 128
        for e in range(2):
            pos = np.nonzero((tokens % 2) == e)[0]
            assert len(pos) <= cap, f"parity capacity exceeded: {len(pos)} > {cap}"
            pid = (tokens[pos] // 2).astype(np.int16)
            pad = np.zeros(cap - len(pos), np.int16)
            idx_cols.append(_wrap_idx(np.concatenate([pid, pad])))
            i = np.arange(len(pos))
            s, p = i // 128, i % 128
            memb[p, slab_base + e * nsl + s, col_fn(pos, s)] = w

    for t, key in enumerate(SEQ_KEYS):
        toks = np.asarray(inputs[key][sl], dtype=np.int64).reshape(-1)

        def col_fn(pos, s):
            b, node = pos // LQ, (pos % LQ) // npn
            se = (s // 2) * 2 if DR else s
            base = np.clip(span * se - (WB - span) // 2, 0, Bc - WB)
            db = b - base
            assert (db >= 0).all() and (db < WB).all(), "seq window violated"
            return db * 4 + node

        add_stream(toks, CAP_S, memb_s, t * 2 * SL_S, col_fn, 1.0 / npn)

    for k, key in enumerate(ATTR_KEYS):
        toks = np.asarray(inputs[key][sl], dtype=np.int64).reshape(-1)
        add_stream(toks, CAP_A, memb_a, k * 2 * SL_A,
                   lambda pos, s: pos // L, 1.0 / L)

    f32 = lambda k: np.asarray(inputs[k], dtype=np.float32)
    bf16 = lambda a: np.ascontiguousarray(a.astype(ml_dtypes.bfloat16))
    if "_emb_fp8" not in inputs:
        inputs["_emb_fp8"] = np.ascontiguousarray(
            (np.asarray(inputs["emb"], dtype=np.float32) * S_EMB).astype(FP8NP))
        inputs["_w_cache"] = {
            "Wioux": bf16(f32("Wioux") / S_EMB),
            "Wiouh": bf16(f32("Wiouh")),
            "Wfx": bf16(f32("Wfx") / S_EMB),
            "Wfh": bf16(f32("Wfh")),
            "Wwh": bf16(f32("Wwh")),
            "wsum": np.ascontiguousarray(f32("Wwp").sum(axis=1)),
            "biou": f32("bioux") + f32("biouh"),
            "bf": f32("bfx") + f32("bfh"),
            "bwh": np.ascontiguousarray(f32("bwh")),
        }
    return {
        "emb": inputs["_emb_fp8"],
        "idx": np.ascontiguousarray(np.concatenate(idx_cols, axis=1)),
        "memb_s": memb_s,
        "memb_a": memb_a,
        **inputs["_w_cache"],
    }


def kernel(**inputs) -> np.ndarray:
    Bc = B // NC_CORES
    CAP_A, CAP_S = _parity_caps(inputs)
    nc = _get_program(Bc, L, LQ, V, D, M, H, O, CAP_A, CAP_S, REPS)
    in_maps = [_prep_core_inputs(inputs, ci, Bc, L, LQ, CAP_A, CAP_S)
               for ci in range(NC_CORES)]
    res = run_bass_kernel_spmd(nc, in_maps, core_ids=list(range(NC_CORES)))
    return np.concatenate([res.results[ci]["out"] for ci in range(NC_CORES)])
